# revision 11
# baseline (speedup 1.0000x reference)
"""Trainium2 Bass kernel for nn_CE2FlowOperator (flow recurrence, 10 steps).

Strategy: pure data parallel over the flattened (B*S)=131072 row dimension,
16384 rows per core on 8 cores. On-chip layout keeps H=128 on SBUF partitions
and rows on the free dimension, processed in tiles of R=512 rows that run all
10 flow steps without touching HBM (read x once, write out once).

Numerics: the mask threshold prob>0.5 makes the recurrence brutally
sensitive (0.8% of rows sit within 1e-3 of the threshold per step), so the
mask path (wm1, wm2) and decoder (wd, wdd) matmuls reproduce fp32 via the
3-pass f32r hi/lo split (f32r = 11-bit mantissa storage; hi*hi products are
exact in fp32 PSUM). The three encoder matmuls only feed the state through
a 0.1-scaled delta, which an 11-bit single-pass f32r tolerates (measured
~1e-5 end-to-end error, no mask flips) - 16 PE passes/step vs 22 for
all-split.

Per step (R=512-row tile; engine assignment balances ACT/DVE/Pool):
    sh   = f32r(state)                                     [DVE copy]
    enc1 = sh @ (0.1*We1); enc2 = sh @ We2; gz = sh @ Wg   [PE 1-pass each]
    gate = sigmoid(gz + bg)                                [ACT]
    dirng = (enc2 + be2) * gate                            [DVE stt]
    tanhd = tanh(dirng)                                    [ACT]
    magg = (enc1 + 0.1*be1) * gate                         [Pool stt]
    tmul = magg * tanhd                                    [DVE]
    new  = tmul + state                                    [Pool stt]
    nh   = f32r(new); nl = new - nh                        [DVE copy; Pool stt]
    m1p  = split_mm(nh, nl, Wm1); hid = relu(m1p + bm1)    [PE x3; ACT]
    zb   = hid @ (Wm2 broadcast to 128 cols), fp32         [PE x4]
    vh   = (zb > -bm2)*nh; vl = (zb > -bm2)*nl             [DVE stt; Pool stt]
    statep = split_mm(nh,nl, Wd) + split_mm(vh,vl, Wdd)    [PE x6, PSUM accum]
    state  = statep + bd                                   [ACT Identity+bias]
"""

import numpy as np
from contextlib import ExitStack

import concourse.bacc as bacc
import concourse.bass as bass
import concourse.mybir as mybir
import concourse.tile as tile
import concourse.bass_isa as bass_isa
from concourse import bass_utils

F32 = mybir.dt.float32
F32R = mybir.dt.float32r
AF = mybir.ActivationFunctionType
ALU = mybir.AluOpType

H = 128
B, S = 64, 2048
N = B * S          # 131072 rows
NCORES = 8
PER = N // NCORES  # 16384 rows per core
R = 512            # rows per tile (one PSUM bank of fp32)
NT = PER // R      # 32 tiles per core
STEPS = 10
SIG_T0 = 8.9407e-08   # fl32(sigmoid(z)) > 0.5  <=>  z > t0

_CACHE = {}


def _build(bm2_val: float, G=32, SB_BUFS=3, ST_BUFS=38):
    nc = bacc.Bacc("TRN2", target_bir_lowering=False, debug=False,
                   num_devices=NCORES)

    x_d = nc.dram_tensor("x", [H, PER], F32, kind="ExternalInput")
    out_d = nc.dram_tensor("out", [H, PER], F32, kind="ExternalOutput")
    we1_d = nc.dram_tensor("we1", [H, H], F32, kind="ExternalInput")
    we2_d = nc.dram_tensor("we2", [H, H], F32, kind="ExternalInput")
    wg_d = nc.dram_tensor("wg", [H, H], F32, kind="ExternalInput")
    wm1_d = nc.dram_tensor("wm1", [H, 64], F32, kind="ExternalInput")
    wm2r_d = nc.dram_tensor("wm2r", [64, H], F32, kind="ExternalInput")
    wd_d = nc.dram_tensor("wd", [H, H], F32, kind="ExternalInput")
    wdd_d = nc.dram_tensor("wdd", [H, H], F32, kind="ExternalInput")
    be1_d = nc.dram_tensor("be1", [H, 1], F32, kind="ExternalInput")
    be2_d = nc.dram_tensor("be2", [H, 1], F32, kind="ExternalInput")
    bg_d = nc.dram_tensor("bg", [H, 1], F32, kind="ExternalInput")
    bm1_d = nc.dram_tensor("bm1", [64, 1], F32, kind="ExternalInput")
    bd_d = nc.dram_tensor("bd", [H, 1], F32, kind="ExternalInput")

    with tile.TileContext(nc) as tc, ExitStack() as ctx:
        wp = ctx.enter_context(tc.tile_pool(name="weights", bufs=1))
        sb = ctx.enter_context(tc.tile_pool(name="data", bufs=SB_BUFS))
        nhp = ctx.enter_context(tc.tile_pool(name="nhl", bufs=6))
        sp = ctx.enter_context(tc.tile_pool(name="states", bufs=ST_BUFS))
        ps = ctx.enter_context(tc.tile_pool(name="psum", bufs=1, space="PSUM"))
        ps2 = ctx.enter_context(tc.tile_pool(name="psum2", bufs=2,
                                             space="PSUM"))

        we1 = wp.tile([H, H], F32)
        we2 = wp.tile([H, H], F32)
        wg = wp.tile([H, H], F32)
        wm1 = wp.tile([H, 64], F32)
        wm2r = wp.tile([64, H], F32)
        wd = wp.tile([H, H], F32)
        wdd = wp.tile([H, H], F32)
        be1 = wp.tile([H, 1], F32)
        be2 = wp.tile([H, 1], F32)
        bg = wp.tile([H, 1], F32)
        bm1 = wp.tile([64, 1], F32)
        bd = wp.tile([H, 1], F32)
        for t_, d_ in ((we1, we1_d), (we2, we2_d), (wg, wg_d), (wm1, wm1_d),
                       (wm2r, wm2r_d), (wd, wd_d), (wdd, wdd_d),
                       (be1, be1_d), (be2, be2_d),
                       (bg, bg_d), (bm1, bm1_d), (bd, bd_d)):
            nc.sync.dma_start(t_[:], d_[:])

        # encoder weights: single-pass f32r (11-bit rounded at copy)
        enc_w = {}
        for nm, w in (("we1", we1), ("we2", we2), ("wg", wg)):
            wr = wp.tile([H, H], F32R, tag=f"wr_{nm}")
            nc.vector.tensor_copy(wr[:], w[:])
            enc_w[nm] = wr

        # mask/decoder weights: hi/lo split for the exact 3-pass scheme
        wsplit = {}
        for nm, w in (("wm1", wm1), ("wd", wd), ("wdd", wdd)):
            shape = [H, 64] if nm == "wm1" else [H, H]
            w_hi = wp.tile(shape, F32R, tag=f"whi_{nm}")
            nc.vector.tensor_copy(w_hi[:], w[:])
            w_lo = wp.tile(shape, F32R, tag=f"wlo_{nm}")
            nc.vector.scalar_tensor_tensor(
                w_lo[:], w[:], 0.0, w_hi[:], ALU.add, ALU.subtract)
            wsplit[nm] = (w_hi, w_lo)

        def split_mm(out_, nm, rhs_hi, rhs_lo, start=True, stop=True):
            w_hi, w_lo = wsplit[nm]
            nc.tensor.matmul(out_[:], w_hi[:], rhs_hi[:],
                             start=start, stop=False)
            nc.tensor.matmul(out_[:], w_hi[:], rhs_lo[:],
                             start=False, stop=False)
            nc.tensor.matmul(out_[:], w_lo[:], rhs_hi[:],
                             start=False, stop=stop)

        # --- 9-stage software pipeline over flat (step, tile) iterations ---
        # lag 0: sh [DVE], enc x3 [PE], gate [ACT], dirng/magg [DVE]
        # lag 1: tanh [ACT]
        # lag 2: tmul, new [Pool]
        # lag 3: nh [DVE], nl [Pool]
        # lag 4: wm1 x3 [PE]
        # lag 5: hid [ACT]
        # lag 6: wm2 [PE]
        # lag 7: wd x3, wdd x3 [PE], vh/vl [DVE]
        # lag 8: evac [ACT] (+ output DMA on the last step)
        # PSUM banks: enc1p/enc2p/gzp (1 each), m1p (2), zbp (2), statep (1)
        states = {}
        for it in range(NT):
            state = sp.tile([H, R], F32, tag="state")
            nc.sync.dma_start(state[:], x_d[:, it * R:(it + 1) * R])
            states[it] = state

        thresh = float(-bm2_val) + SIG_T0
        ctxs = {}          # flat index -> dict of live tiles
        sh_pre = {}        # flat index -> f32r state copy (made 1 iter early)
        total = STEPS * NT

        def emit_sh(m):
            step, it = divmod(m, NT)
            sh = sb.tile([H, R], F32R, tag="sh")
            nc.vector.tensor_copy(sh[:], states[it][:])
            sh_pre[m] = sh

        for n in range(total + 8):
            # ---- lag 8: evacuate state (+ output DMA) -------------------
            if n >= 8:
                c = ctxs[n - 8]
                state = sp.tile([H, R], F32, tag="state")
                nc.scalar.activation(state[:], c["statep"][:], AF.Identity,
                                     bias=bd[:])
                states[c["tile"]] = state
                if c["step"] == STEPS - 1:
                    it = c["tile"]
                    nc.sync.dma_start(out_d[:, it * R:(it + 1) * R], state[:])
                del ctxs[n - 8]

            # ---- lag 0: encoder matmuls (sh prefetched last iter) -------
            if n < total:
                step, it = divmod(n, NT)
                c = {"step": step, "tile": it, "state": states[it]}
                ctxs[n] = c
                if n == 0:
                    emit_sh(0)
                sh = sh_pre.pop(n)
                if n + 1 < total:
                    emit_sh(n + 1)
                enc1p = ps.tile([H, R], F32, tag="enc1p")
                enc2p = ps.tile([H, R], F32, tag="enc2p")
                gzp = ps.tile([H, R], F32, tag="gzp")
                nc.tensor.matmul(enc1p[:], enc_w["we1"][:], sh[:],
                                 start=True, stop=True)
                nc.tensor.matmul(enc2p[:], enc_w["we2"][:], sh[:],
                                 start=True, stop=True)
                nc.tensor.matmul(gzp[:], enc_w["wg"][:], sh[:],
                                 start=True, stop=True)

            # ---- lag 7 (DVE part): mask-select halves -------------------
            if 7 <= n < total + 7:
                c = ctxs[n - 7]
                vh = sb.tile([H, R], F32R, tag="vh")
                nc.vector.scalar_tensor_tensor(
                    vh[:], c["zbp"][:], thresh, c["nh"][:],
                    ALU.is_gt, ALU.mult)
                vl = sb.tile([H, R], F32R, tag="vl")
                nc.vector.scalar_tensor_tensor(
                    vl[:], c["zbp"][:], thresh, c["nl"][:],
                    ALU.is_gt, ALU.mult)
                c["vh"], c["vl"] = vh, vl

            # ---- lag 3 (DVE part): hi half of new -----------------------
            if 3 <= n < total + 3:
                c = ctxs[n - 3]
                nh = nhp.tile([H, R], F32R, tag="nh")
                nc.vector.tensor_copy(nh[:], c["new"][:])
                c["nh"] = nh

            # ---- lag 3 (Pool part): lo half -----------------------------
            if 3 <= n < total + 3:
                c = ctxs[n - 3]
                nl = nhp.tile([H, R], F32R, tag="nl")
                nc.gpsimd.tensor_tensor(nl[:], c["new"][:], c["nh"][:],
                                        ALU.subtract)
                c["nl"] = nl

            # ---- lag 0 (ACT): gate --------------------------------------
            if n < total:
                c = ctxs[n]
                gate = sb.tile([H, R], F32, tag="gate")
                nc.scalar.activation(gate[:], gzp[:], AF.Sigmoid, bias=bg[:])
                c["gate"] = gate

            # ---- lag 4: mirror-detector layer 1 -------------------------
            if 4 <= n < total + 4:
                c = ctxs[n - 4]
                m1p = ps2.tile([64, R], F32, tag="m1p")
                split_mm(m1p, "wm1", c["nh"], c["nl"])
                c["m1p"] = m1p

            # ---- lag 5 (ACT): relu --------------------------------------
            if 5 <= n < total + 5:
                c = ctxs[n - 5]
                hid = sb.tile([64, R], F32, tag="hid")
                nc.scalar.activation(hid[:], c["m1p"][:], AF.Relu,
                                     bias=bm1[:])
                c["hid"] = hid
                del c["m1p"]

            # ---- lag 6 (PE): z broadcast matmul -------------------------
            if 6 <= n < total + 6:
                c = ctxs[n - 6]
                zbp = ps2.tile([H, R], F32, tag="zbp")
                nc.tensor.matmul(zbp[:], wm2r[:], c["hid"][:])
                c["zbp"] = zbp

            # ---- lag 0 (DVE): gated encoder halves ----------------------
            if n < total:
                c = ctxs[n]
                dirng = sb.tile([H, R], F32, tag="dirng")
                nc.vector.scalar_tensor_tensor(
                    dirng[:], enc2p[:], be2[:], c["gate"][:],
                    ALU.add, ALU.mult)
                magg = sb.tile([H, R], F32, tag="magg")
                nc.vector.scalar_tensor_tensor(
                    magg[:], enc1p[:], be1[:], c["gate"][:],
                    ALU.add, ALU.mult)
                c["dirng"], c["magg"] = dirng, magg

            # ---- lag 1 (ACT): tanh --------------------------------------
            if 1 <= n < total + 1:
                c = ctxs[n - 1]
                tanhd = sb.tile([H, R], F32, tag="tanhd")
                nc.scalar.activation(tanhd[:], c["dirng"][:], AF.Tanh)
                c["tanhd"] = tanhd

            # ---- lag 2 (Pool): delta + new state ------------------------
            if 2 <= n < total + 2:
                c = ctxs[n - 2]
                tmul = sb.tile([H, R], F32, tag="tmul")
                nc.gpsimd.tensor_tensor(tmul[:], c["magg"][:],
                                        c["tanhd"][:], ALU.mult)
                new = sb.tile([H, R], F32, tag="new")
                nc.gpsimd.tensor_tensor(new[:], tmul[:], c["state"][:],
                                        ALU.add)
                c["new"] = new

            # ---- lag 7 (PE): decoder matmuls ----------------------------
            if 7 <= n < total + 7:
                c = ctxs[n - 7]
                statep = ps.tile([H, R], F32, tag="statep")
                split_mm(statep, "wd", c["nh"], c["nl"],
                         start=True, stop=False)
                split_mm(statep, "wdd", c["vh"], c["vl"],
                         start=False, stop=True)
                c["statep"] = statep

    nc.compile()
    return nc


def kernel(x, We, be, Wg, bg, Wm1, bm1, Wm2, bm2, Wd, bd):
    x = np.ascontiguousarray(np.asarray(x, dtype=np.float32))
    We = np.asarray(We, dtype=np.float32)
    be = np.asarray(be, dtype=np.float32)
    Wg_ = np.asarray(Wg, dtype=np.float32)
    bg_ = np.asarray(bg, dtype=np.float32)
    Wm1_ = np.asarray(Wm1, dtype=np.float32)
    bm1_ = np.asarray(bm1, dtype=np.float32)
    Wm2_ = np.asarray(Wm2, dtype=np.float32)
    bm2_ = np.asarray(bm2, dtype=np.float32)
    Wd_ = np.asarray(Wd, dtype=np.float32)
    bd_ = np.asarray(bd, dtype=np.float32)

    bm2_val = float(bm2_.reshape(-1)[0])
    key = ("v5", bm2_val)
    if key not in _CACHE:
        _CACHE[key] = _build(bm2_val)
    nc = _CACHE[key]

    wd_h = np.ascontiguousarray(Wd_[:H])                   # (H, H)
    wdd = np.ascontiguousarray(wd_h[::-1] - wd_h)          # flip(Wd) - Wd
    weights = {
        "we1": np.ascontiguousarray(0.1 * We[:, :H]),
        "we2": np.ascontiguousarray(We[:, H:]),
        "wg": Wg_,
        "wm1": Wm1_,
        "wm2r": np.ascontiguousarray(np.tile(Wm2_.reshape(64, 1), (1, H))),
        "wd": wd_h,
        "wdd": wdd,
        "be1": (0.1 * be[:H]).reshape(H, 1),
        "be2": be[H:].reshape(H, 1),
        "bg": bg_.reshape(H, 1),
        "bm1": bm1_.reshape(64, 1),
        "bd": bd_.reshape(H, 1),
    }
    weights = {k: np.ascontiguousarray(v.astype(np.float32))
               for k, v in weights.items()}

    xf = x.reshape(N, H)
    in_maps = []
    for c in range(NCORES):
        m = {"x": np.ascontiguousarray(xf[c * PER:(c + 1) * PER].T)}
        m.update(weights)
        in_maps.append(m)

    res = bass_utils.run_bass_kernel_spmd(nc, in_maps,
                                          core_ids=list(range(NCORES)))
    out = np.concatenate([res.results[c]["out"].T for c in range(NCORES)],
                         axis=0)
    return out.reshape(B, S, H)
